# revision 16
# baseline (speedup 1.0000x reference)
"""Self-contained Trainium2 Bass kernel for nn_DualGATv2 (3-layer GATv2 + MLP).

V3. Sharding: nodes dealt across 8 NeuronCores into 49 blocks of 8x128,
grouped by (in-degree, forced-window-count) through a 6-round fixed-point
iteration so each block's padded slot count is near the max-degree lower
bound; weights replicated. Per-layer projected source features live in a
bf16 HBM table built via two strided AllGathers (block-interleaved row
layout with rotation so the pad/poison block is reachable from both int16
index windows). Per-edge messages are fetched with whole-block gpsimd
dma_gather calls; self-loops never touch DMA - each block's last slot is
filled from the locally-projected xl via a vector copy.

Edge math per 128-dst block (dst-major, k = padded incoming-edge slots):
  fused custom DVE op computes t = 3z + 2|z| (z = msg + xr) equal to
  leakyrelu(z)*6/(1+s) with the constant folded into att; att-mult at DVE
  2x; logits reduce per (k,head); scatter-softmax is exp() without
  max-subtraction (logits bounded, pad slots gather poison rows that force
  logits ~ -1e5); alpha-weighted aggregation in contiguous (k,h,d) layout.
  Per-layer tails (1/denominator, LayerNorm, ELU, residual) are batched
  across blocks, and chunk-0 tails/projections/AllGather are issued mid-
  layer so they overlap the remaining edge blocks' gathers.
"""
import sys
import numpy as np

sys.path.insert(0, '/opt/trn_rl_repo')

import concourse.bass as bass
import concourse.bacc as bacc
import concourse.tile as tile
from concourse import mybir, library_config
from concourse import bass_utils
from concourse._compat import cdiv

F32 = mybir.dt.float32
BF16 = mybir.dt.bfloat16
I16 = mybir.dt.int16
AL = mybir.AluOpType
ACTF = mybir.ActivationFunctionType
AX = mybir.AxisListType

NC = 8
P = 128
HID = 32
HEADS = 4
NEG_SLOPE = 0.2
LN_EPS = 1e-5
NEG_BIG = -1.0e30
IDX_WIN = 32768

# ----------------------------------------------------------------------------
# custom DVE ops (registered through the documented dve_ops extension point)
# ----------------------------------------------------------------------------

_DVE_REGISTERED = {}


def _register_dve_ops():
    if _DVE_REGISTERED:
        return _DVE_REGISTERED
    from concourse import dve_ops
    from concourse.dve_spec import (Spec, Src0, Src1, C0, Zero, maxx, select,
                                    lower)
    from concourse.dve_uop import DveOpSpec

    def _mk(name, spec):
        for op in dve_ops.OPS:
            if op.name == name:
                _DVE_REGISTERED[name] = op
                return op
        shas = {}
        for ver in ('v3', 'v4'):
            tmp = DveOpSpec(name=name, opcode=1, uops=lower(spec, ver=ver))
            shas[ver] = tmp.sha(ver)
        op = dve_ops.DveOp(name, spec, subdim=False, uops_sha=shas)
        dve_ops.OPS.append(op)
        dve_ops.CUSTOM_DVE_SPECS[name] = spec
        dve_ops._SUB_OPCODE_FOR_NAME[name] = (
            dve_ops._CUSTOM_DVE_ROW_BASE + len(dve_ops.OPS) - 1)
        assert dve_ops._SUB_OPCODE_FOR_NAME[name] < 0x20
        _DVE_REGISTERED[name] = op
        return op

    # t = 3*z + 2*|z| (single stream, no scalar slots).  For slope 0.2 this
    # equals leakyrelu(z) * 2/(1+s) * 3, folded into the att constant.
    def _lrelu_ref(in0, in1, s0, s1, imm2):
        z = in0.astype(np.float32)
        return 3.0 * z + 2.0 * np.abs(z)

    _z = Src0
    _a = maxx(_z, Zero - _z)
    _mk('GAT_LRELU_ANT', Spec(
        body=((_z + _z) + _z) + (_a + _a),
        reference=_lrelu_ref,
    ))
    # two-stream variant: z = in0 + in1 computed inline (no scalar slots)
    def _lrelu2_ref(in0, in1, s0, s1, imm2):
        z = in0.astype(np.float32) + \
            np.asarray(in1, np.float32).reshape(in0.shape)
        return 3.0 * z + 2.0 * np.abs(z)

    _z2 = Src0 + Src1
    _a2 = maxx(_z2, Zero - _z2)
    _mk('GAT_LRELU2_ANT', Spec(
        body=((_z2 + _z2) + _z2) + (_a2 + _a2),
        reference=_lrelu2_ref,
    ))
    # elu select: out = in0 if in0 > 0 else in1 + c0   (in1 = exp(in0))
    def _elusel_ref(in0, in1, s0, s1, imm2):
        i1 = np.asarray(in1, np.float32).reshape(in0.shape)
        return np.where(in0 > 0, in0.astype(np.float32), i1 + s0)

    _mk('GAT_ELUSEL_ANT', Spec(
        body=select(Src0 > Zero, Src0, Src1 + C0),
        reference=_elusel_ref,
    ))
    return _DVE_REGISTERED


# ----------------------------------------------------------------------------
# host-side preprocessing
# ----------------------------------------------------------------------------

def _prep(x, edge_index):
    x = np.asarray(x, dtype=np.float32)
    N = x.shape[0]
    # self-loops are handled locally in the kernel (slot kt of each block is
    # filled from the locally-projected xl), so only real edges are gathered.
    src = np.asarray(edge_index[0], dtype=np.int64)
    dst = np.asarray(edge_index[1], dtype=np.int64)

    deg = np.bincount(dst, minlength=N)       # in-degree, no self loop

    NSH_REAL = cdiv(N, NC)
    NSH = cdiv(NSH_REAL + 1, P) * P       # >= 1 pad row per core
    NBLK = NSH // P
    TAB = NC * NSH
    # Table rows are block-interleaved with rotation ROT: node at
    # (core, pos=b*128+p) lives at table row ((b+ROT)%NBLK * NC + core)*P+p.
    # Built by TWO strided AllGather chunks with no wraparound:
    #   chunk0 = blocks [0, NBLK-ROT) -> row-blocks [ROT, NBLK)
    #   chunk1 = blocks [NBLK-ROT, NBLK) -> row-blocks [0, ROT)
    # ROT=20 puts the pad block (NBLK-1) at row-block 19, inside the flex
    # region [HI_BASE, W_LO) so both windows can reach the poison rows.
    ROT = 20
    CH0_BLK = NBLK - ROT
    W_LO = min(IDX_WIN, TAB)
    HI_BASE = max(0, TAB - IDX_WIN)

    def place(order):
        ranks = np.arange(N)
        g, j = ranks // NC, ranks % NC
        core_of_rank = np.where(g % 2 == 0, j, NC - 1 - j)
        core = np.zeros(N, dtype=np.int64)
        core[order] = core_of_rank
        pos = np.zeros(N, dtype=np.int64)
        pos[order] = g
        return core, pos

    def rows_of(core, pos):
        b = pos // P
        p = pos % P
        rb = (b + ROT) % NBLK
        return (rb * NC + core) * P + p

    # Iterated placement: regroup nodes by (in-degree, forced-A count) so
    # each 1024-node block is homogeneous in both -> minimal slot padding.
    order = np.argsort(-deg, kind='stable')
    for it in range(6):
        core, pos = place(order)
        row = rows_of(core, pos)
        rs = row[src]
        fA = np.bincount(dst[rs < HI_BASE], minlength=N)   # forced A
        fB = np.bincount(dst[rs >= W_LO], minlength=N)     # forced B
        if it < 5:
            order = np.lexsort((-fA, -deg))

    pad_p0 = NSH_REAL - (NBLK - 1) * P
    PZ_LO = int(rows_of(np.int64(1), np.int64(NSH_REAL)))
    PZ_HI = int(rows_of(np.int64(6), np.int64(NSH_REAL))) - HI_BASE
    assert 0 <= PZ_LO < W_LO and 0 <= PZ_HI < IDX_WIN

    e_order = np.argsort(dst, kind='stable')
    src_s = src[e_order]
    dst_s = dst[e_order]
    starts = np.searchsorted(dst_s, np.arange(N))
    ends = np.searchsorted(dst_s, np.arange(N) + 1)
    rs_all = row[src_s]

    node_at = np.full((NC, NSH), -1, dtype=np.int64)
    node_at[core, pos] = np.arange(N)

    # exact per-block split: ka = max forced-A, kb = max(max forced-B,
    # maxd - ka); per-node A-count ca = min(ka, d - forced-B).
    K_A = np.zeros(NBLK, dtype=np.int64)
    K_B = np.zeros(NBLK, dtype=np.int64)
    ca_of = np.zeros(N, dtype=np.int64)
    for b in range(NBLK):
        sl = node_at[:, b * P:(b + 1) * P].reshape(-1)
        sl = sl[sl >= 0]
        if not len(sl):
            K_A[b] = K_B[b] = 0
            continue
        d = deg[sl]
        ka = int(fA[sl].max())
        kb = max(int(fB[sl].max()), int(d.max()) - ka)
        K_A[b] = ka
        K_B[b] = kb
        ca_of[sl] = np.minimum(ka, d - fB[sl])

    SUMKT = int((K_A + K_B).sum())
    IDXW = int(8 * SUMKT)
    idx_all = np.zeros((NC, P, IDXW), dtype=np.int16)

    def wrap(flat):
        n = len(flat)
        S = cdiv(n, 16)
        a = np.zeros(16 * S, np.int16)
        a[:n] = flat
        return np.tile(a.reshape(S, 16).T, (8, 1))

    icol = 0
    for b in range(NBLK):
        ka, kb = int(K_A[b]), int(K_B[b])
        for c in range(NC):
            flatA = np.full(ka * P, PZ_LO, np.int64)
            flatB = np.full(kb * P, PZ_HI, np.int64)
            for p in range(P):
                n = node_at[c, b * P + p]
                if n < 0:
                    continue  # pad partition: denom comes from the self slot
                rs = rs_all[starts[n]:ends[n]]
                inA = rs < HI_BASE
                flex = ~inA & (rs < W_LO)
                ca = int(ca_of[n])
                nfa = int(inA.sum())
                selA = inA.copy()
                if ca > nfa:
                    fidx = np.nonzero(flex)[0]
                    selA[fidx[:ca - nfa]] = True
                ea = rs[selA]
                eb = rs[~selA] - HI_BASE
                la, lb = len(ea), len(eb)
                if la:
                    flatA[np.arange(la) * P + p] = ea
                if lb:
                    flatB[np.arange(lb) * P + p] = eb
            if ka:
                idx_all[c, :, icol:icol + 8 * ka] = \
                    wrap(flatA.astype(np.int16))
            if kb:
                idx_all[c, :, icol + 8 * ka:icol + 8 * (ka + kb)] = \
                    wrap(flatB.astype(np.int16))
        icol += 8 * (ka + kb)

    IND = x.shape[1]
    xT_own = np.zeros((NC, IND, NSH), dtype=np.float32)
    xT_own[core, :, pos] = x                            # [NC, IND, NSH]

    padmask = (np.arange(P) < pad_p0).astype(np.float32).reshape(P, 1)
    orow = core * NSH + pos                  # output unshard (pos-major)
    st = dict(N=N, NSH=NSH, NSH_REAL=NSH_REAL, NBLK=NBLK, TAB=TAB,
              W_LO=W_LO, HI_BASE=HI_BASE, K_A=K_A.tolist(),
              K_B=K_B.tolist(), SUMKT=SUMKT, IDXW=IDXW, IN_DIM=IND,
              CH0_BLK=CH0_BLK, ROT=ROT)
    return st, xT_own, idx_all, orow, padmask


def _blkdiag(w, n):
    import ml_dtypes
    r, c = w.shape
    out = np.zeros((r * n, c * n), np.float32)
    for i in range(n):
        out[i * r:(i + 1) * r, i * c:(i + 1) * c] = w
    return np.ascontiguousarray(out).astype(ml_dtypes.bfloat16)


def _rep(v):
    v = np.asarray(v, dtype=np.float32).reshape(1, -1)
    return np.ascontiguousarray(np.tile(v, (P, 1)))


# ----------------------------------------------------------------------------
# kernel builder
# ----------------------------------------------------------------------------

def _build(st):
    import os
    GMAX = int(os.environ.get('GMAX', '7'))      # ring caps a call at 57 descs/DMA
    NO_CUSTOM = os.environ.get('NO_CUSTOM', '0') == '1'
    ALTW = os.environ.get('ALTW', '1') == '1'
    _register_dve_ops()
    from concourse.dve_ops import OPS as _OPS
    LRELU_OP = next(o for o in _OPS if o.name == 'GAT_LRELU_ANT')
    LRELU2_OP = next(o for o in _OPS if o.name == 'GAT_LRELU2_ANT')
    FUSE2 = os.environ.get('FUSE2', '1') == '1'
    ELUSEL_OP = next(o for o in _OPS if o.name == 'GAT_ELUSEL_ANT')

    NSH, NBLK, TAB = st['NSH'], st['NBLK'], st['TAB']
    NSH_REAL = st['NSH_REAL']
    W_LO, HI_BASE = st['W_LO'], st['HI_BASE']
    K_A, K_B = st['K_A'], st['K_B']
    SUMKT = st['SUMKT']
    IDXW = st['IDXW']
    IND = st['IN_DIM']
    ROT = st['ROT']
    KT = [K_A[b] + K_B[b] for b in range(NBLK)]     # gathered edge slots
    KTMAX = max(KT) + 1                             # +1 local self slot
    TRIV = st['trivial']  # biases zero, gamma ones => skip those ops

    # layer cfg: (heads, dout, feat, in_feat)
    LCFG = [(HEADS, HID, HEADS * HID, IND),
            (HEADS, HID, HEADS * HID, HEADS * HID),
            (1, HID, HID, HEADS * HID)]

    SCRATCH = int(os.environ.get('SCRATCH', '16384'))
    nc = bacc.Bacc('TRN2', target_bir_lowering=False, debug=False,
                   enable_asserts=True, num_devices=NC,
                   num_swdge_queues=4,
                   dynamic_dma_scratch_size=SCRATCH)

    def ein(name, shape, dt=F32):
        return nc.dram_tensor(name, shape, dt, kind='ExternalInput')

    xTo_d = ein('xT_own', [IND, NSH], BF16)
    idx_d = ein('idx_all', [P, IDXW], I16)
    pmask_d = ein('padmask', [P, 1])
    WC_d = [ein(f'wc_{l}', [P, 128], BF16) for l in range(3)]
    W01_d = [ein('W01_0', [IND, 256], BF16),
             ein('W01_1', [128, 256], BF16),
             ein('W01_2', [128, 64], BF16)]
    ATTB_d = [ein('attb_0', [P, KTMAX * 128], BF16),
              ein('attb_1', [P, KTMAX * 128], BF16),
              ein('attb_2', [P, KTMAX * 32], BF16)]
    cW1_d = ein('cW1', [96, 48], BF16)
    cW2_d = ein('cW2', [96, 6], BF16)
    ident_d = ein('ident', [P, P], BF16)
    if not TRIV:
        BL01_d = [ein('bl01_0', [P, 256]), ein('bl01_1', [P, 256]),
                  ein('bl01_2', [P, 64])]
        GG_d = [ein('g_0', [P, 128]), ein('g_1', [P, 128]),
                ein('g_2', [P, 32])]
        BE_d = [ein('be_0', [P, 128]), ein('be_1', [P, 128]),
                ein('be_2', [P, 32])]
        BO_d = [ein('bo_0', [P, 128]), ein('bo_1', [P, 128]),
                ein('bo_2', [P, 32])]
        cb1_d = ein('cb1', [P, 48])
        cb2_d = ein('cb2', [P, 1])
    out_d = nc.dram_tensor('out', [NSH], F32, kind='ExternalOutput')

    tabs = [nc.dram_tensor(f'table{l}', [TAB, 128], BF16, kind='Internal')
            for l in range(3)]
    tab_stage = [nc.dram_tensor(f'tstage{l}', [TAB, 128], BF16,
                                kind='Internal', addr_space='Shared')
                 for l in range(3)]
    ag_in = [nc.dram_tensor(f'ag_in{l}', [NSH, 128], BF16, kind='Internal')
             for l in range(3)]

    import contextlib
    with tile.TileContext(nc) as tc, contextlib.ExitStack() as ctx:
        cpool = ctx.enter_context(tc.tile_pool(name='consts', bufs=1))
        attpool = ctx.enter_context(tc.tile_pool(name='att', bufs=1))
        gpool = ctx.enter_context(tc.tile_pool(name='g', bufs=3))
        tpool = ctx.enter_context(tc.tile_pool(name='t', bufs=2))
        spool = ctx.enter_context(tc.tile_pool(name='small', bufs=3))
        npool = ctx.enter_context(tc.tile_pool(name='node', bufs=1))
        hpool = ctx.enter_context(tc.tile_pool(name='h', bufs=1))
        xpool = ctx.enter_context(tc.tile_pool(name='xt', bufs=4))
        stpool = ctx.enter_context(tc.tile_pool(name='stage', bufs=4))
        pspool = ctx.enter_context(tc.tile_pool(name='ps', bufs=3,
                                                space='PSUM'))
        ps2pool = ctx.enter_context(tc.tile_pool(name='ps2', bufs=2,
                                                 space='PSUM'))

        def load_const(pool, dram, shape, dt=F32):
            t = pool.tile(shape, dt, tag='c_' + dram.name,
                          name='c_' + dram.name)
            nc.sync.dma_start(out=t[:], in_=dram[:])
            return t

        ident = load_const(cpool, ident_d, [P, P], BF16)
        W01_s = [load_const(cpool, W01_d[l], list(W01_d[l].shape), BF16)
                 for l in range(3)]
        cW1_s = load_const(cpool, cW1_d, [96, 48], BF16)
        cW2_s = load_const(cpool, cW2_d, [96, 6], BF16)
        if not TRIV:
            BL01_s = [load_const(cpool, BL01_d[l], list(BL01_d[l].shape))
                      for l in range(3)]
            GG_s = [load_const(cpool, GG_d[l], list(GG_d[l].shape))
                    for l in range(3)]
            BE_s = [load_const(cpool, BE_d[l], list(BE_d[l].shape))
                    for l in range(3)]
            BO_s = [load_const(cpool, BO_d[l], list(BO_d[l].shape))
                    for l in range(3)]
            cb1_s = load_const(cpool, cb1_d, [P, 48])
            cb2_s = load_const(cpool, cb2_d, [P, 1])

        pmask_s = load_const(cpool, pmask_d, [P, 1])
        eps_t = cpool.tile([P, 1], F32, tag='eps', name='eps')
        nc.vector.memset(eps_t[:], float(LN_EPS))
        idx_s = cpool.tile([P, IDXW], I16, tag='idx')
        nc.sync.dma_start(out=idx_s[:], in_=idx_d[:])
        WC_s = [load_const(cpool, WC_d[l], [P, 128], BF16)
                for l in range(3)]

        h_res = [hpool.tile([P, NBLK * 128], BF16, tag='h0', name='h0'),
                 hpool.tile([P, NBLK * 128], BF16, tag='h1', name='h1'),
                 hpool.tile([P, NBLK * HID], BF16, tag='h2', name='h2')]
        h_raw = hpool.tile([P, NBLK * 128], F32, tag='hraw')
        xr_res = hpool.tile([P, NBLK * 128], BF16, tag='xr')
        xlo = hpool.tile([P, NBLK * 128], BF16, tag='xlo', name='xlo')
        DN_all = hpool.tile([P, NBLK * HEADS], F32, tag='dn')
        out_sb = hpool.tile([P, NBLK], F32, tag='outsb')

        # per-block idx/mask column offsets
        ic_of, mc_of = [], []
        ic = mc = 0
        for b in range(NBLK):
            ic_of.append(ic)
            mc_of.append(mc)
            ic += 8 * KT[b]
            mc += KT[b]

        qc = [0]

        def edge_block(l, b, att_s):
            H, DO, FE, _ = LCFG[l]
            ka, kb = K_A[b], K_B[b]
            kt = ka + kb          # gathered edge slots
            kt1 = kt + 1          # + local self-loop slot
            G = gpool.tile([P, kt1, 128], BF16, tag='G')
            gm_a = GMAX if GMAX else max(ka, 1)
            gm_b = GMAX if GMAX else max(kb, 1)
            for off in range(0, ka, gm_a):
                kk = min(gm_a, ka - off)
                nc.gpsimd.dma_gather(
                    G[:, off:off + kk, :], tabs[l][0:W_LO, :],
                    idx_s[:, ic_of[b] + 8 * off:ic_of[b] + 8 * (off + kk)],
                    kk * P, kk * P, 128, queue_num=qc[0] % 4)
                qc[0] += 1
            for off in range(0, kb, gm_b):
                kk = min(gm_b, kb - off)
                nc.gpsimd.dma_gather(
                    G[:, ka + off:ka + off + kk, :], tabs[l][HI_BASE:TAB, :],
                    idx_s[:, ic_of[b] + 8 * (ka + off):
                          ic_of[b] + 8 * (ka + off + kk)],
                    kk * P, kk * P, 128, queue_num=qc[0] % 4)
                qc[0] += 1
            # self-loop slot: own projected xl, no gather
            nc.vector.tensor_copy(out=G[:, kt, 0:FE],
                                  in_=xlo[:, b * 128:b * 128 + FE])
            xr_col = xr_res[:, b * FE:(b + 1) * FE]
            T = tpool.tile([P, kt1 * FE], BF16, tag='T')
            T2 = tpool.tile([P, kt1 * FE], BF16, tag='T2')
            T3 = T[:].rearrange('p (k f) -> p k f', k=kt1)
            G3 = G[:, :, 0:FE]
            # t = 3z + 2|z|, z = msg + xr (leaky; 1/3*(1+s)/2 inside att)
            if NO_CUSTOM:
                nc.vector.tensor_tensor(
                    out=T3, in0=G3,
                    in1=xr_col.unsqueeze(1).to_broadcast([P, kt1, FE]),
                    op=AL.add)
                nc.vector.tensor_scalar_mul(
                    T[:], T[:], float(6.0 / (1.0 + NEG_SLOPE)))
                nc.vector.scalar_tensor_tensor(
                    out=T[:], in0=T[:], scalar=float(NEG_SLOPE), in1=T[:],
                    op0=AL.mult, op1=AL.max)
            elif FUSE2:
                nc.vector._custom_dve(
                    LRELU2_OP, out=T3, in0=G3,
                    in1=xr_col.unsqueeze(1).to_broadcast([P, kt1, FE]))
            else:
                nc.vector.tensor_tensor(
                    out=T3, in0=G3,
                    in1=xr_col.unsqueeze(1).to_broadcast([P, kt1, FE]),
                    op=AL.add)
                nc.vector._custom_dve(LRELU_OP, out=T[:], in0=T[:])
            # *= att' (pre-tiled; out-of-place so the 2x bf16 uop applies)
            nc.vector.tensor_tensor(out=T2[:], in0=T[:],
                                    in1=att_s[:, 0:kt1 * FE], op=AL.mult)
            # logits: sum over d per (k,h)
            LG = spool.tile([P, kt1 * H], F32, tag='LG')
            nc.vector.tensor_reduce(
                out=LG[:], in_=T2[:].rearrange('p (kh d) -> p kh d', d=DO),
                axis=AX.X, op=AL.add)
            # softmax numerator (no max subtraction; logits are bounded,
            # pad slots carry poison-row logits of about -1e5)
            A = spool.tile([P, kt1 * H], BF16, tag='A')
            if H == 1:
                nc.scalar.activation(out=A[:], in_=LG[:], func=ACTF.Exp,
                                     accum_out=DN_all[:, b:b + 1])
            else:
                nc.scalar.activation(out=A[:], in_=LG[:], func=ACTF.Exp)
                nc.vector.tensor_reduce(
                    out=DN_all[:, b * H:(b + 1) * H],
                    in_=A[:].rearrange('p (k h) -> p h k', h=H),
                    axis=AX.X, op=AL.add)
            if ALTW:
                # weighted messages in native (k, h, d) layout: mult is
                # fully contiguous (2x bf16); the k-sum is a halving
                # fold-tree of contiguous bf16 adds ping-ponging T/T2
                # (every op out-of-place so the 2x uop applies).
                W = T[:].rearrange('p (k h d) -> p k h d', h=H, d=DO)
                Gv = G[:, :, 0:FE].rearrange('p k (h d) -> p k h d', h=H)
                Av = A[:].rearrange('p (k h) -> p k h', h=H) \
                    .unsqueeze(3).to_broadcast([P, kt1, H, DO])
                nc.vector.tensor_tensor(out=W, in0=Gv, in1=Av, op=AL.mult)
                nc.vector.tensor_reduce(
                    out=h_raw[:, b * FE:(b + 1) * FE],
                    in_=T[:].rearrange('p (k f) -> p f k', f=FE),
                    axis=AX.X, op=AL.add)
            else:
                # weighted messages, (h d k) layout, then sum over k
                if H == 1:
                    W = T[:].rearrange('p (d k) -> p d k', k=kt1)
                    Gv = G[:, :, 0:FE].rearrange('p k d -> p d k')
                    Av = A[:].unsqueeze(1).to_broadcast([P, FE, kt1])
                else:
                    W = T[:].rearrange('p (h d k) -> p h d k', h=H, d=DO)
                    Gv = G[:, :, 0:FE].rearrange('p k (h d) -> p h d k', h=H)
                    Av = A[:].rearrange('p (k h) -> p h k', h=H) \
                        .unsqueeze(2).to_broadcast([P, H, DO, kt1])
                nc.vector.tensor_tensor(out=W, in0=Gv, in1=Av, op=AL.mult)
                nc.vector.tensor_reduce(
                    out=h_raw[:, b * FE:(b + 1) * FE], in_=W,
                    axis=AX.X, op=AL.add)

        def layer_tail(l, b0, b1):
            """Batched over blocks [b0, b1): /denom, LayerNorm, ELU, skip."""
            H, DO, FE, _ = LCFG[l]
            NB = b1 - b0
            NF = NB * FE
            hr = h_raw[:, b0 * FE:b1 * FE]
            # 1/denominator, applied per (block, head)
            R = npool.tile([P, NB * H], F32, tag='R')
            nc.vector.reciprocal_approx_fast(
                R[:], DN_all[:, b0 * H:b1 * H])
            hr4 = hr.rearrange('p (b h d) -> p b h d', h=H, d=DO)
            hr3 = hr.rearrange('p (b f) -> p b f', f=FE)
            nc.vector.tensor_tensor(
                out=hr4, in0=hr4,
                in1=R[:].rearrange('p (b h) -> p b h', h=H)
                .unsqueeze(3).to_broadcast([P, NB, H, DO]),
                op=AL.mult)
            if not TRIV:
                nc.vector.tensor_tensor(
                    out=hr3, in0=hr3,
                    in1=BO_s[l][:, 0:FE].unsqueeze(1)
                    .to_broadcast([P, NB, FE]),
                    op=AL.add)
            # LayerNorm stats, batched per block
            MU = npool.tile([P, NB], F32, tag='MU')
            nc.vector.tensor_reduce(
                out=MU[:], in_=hr3, axis=AX.X, op=AL.add)
            nc.vector.tensor_scalar_mul(MU[:], MU[:], 1.0 / FE)
            SQ = npool.tile([P, NF], BF16, tag='EX')
            nc.vector.tensor_tensor(out=SQ[:], in0=hr, in1=hr, op=AL.mult)
            SSQ = npool.tile([P, NB], F32, tag='SSQ')
            nc.vector.tensor_reduce(
                out=SSQ[:], in_=SQ[:].rearrange('p (b f) -> p b f', f=FE),
                axis=AX.X, op=AL.add)
            MM = npool.tile([P, NB], F32, tag='MM')
            nc.vector.tensor_tensor(out=MM[:], in0=MU[:], in1=MU[:],
                                    op=AL.mult)
            VAR = npool.tile([P, NB], F32, tag='VAR')
            nc.vector.scalar_tensor_tensor(
                out=VAR[:], in0=SSQ[:], scalar=1.0 / FE, in1=MM[:],
                op0=AL.mult, op1=AL.subtract)
            SD = npool.tile([P, NB], F32, tag='SD')
            nc.scalar.activation(out=SD[:], in_=VAR[:], func=ACTF.Sqrt,
                                 bias=eps_t[:])
            IV = npool.tile([P, NB], F32, tag='IV')
            nc.vector.reciprocal_approx_fast(IV[:], SD[:])
            # normalize
            nc.vector.tensor_tensor(
                out=hr3, in0=hr3,
                in1=MU[:].unsqueeze(2).to_broadcast([P, NB, FE]),
                op=AL.subtract)
            nc.vector.tensor_tensor(
                out=hr3, in0=hr3,
                in1=IV[:].unsqueeze(2).to_broadcast([P, NB, FE]),
                op=AL.mult)
            if not TRIV:
                nc.vector.tensor_tensor(
                    out=hr3, in0=hr3,
                    in1=GG_s[l][:, 0:FE].unsqueeze(1)
                    .to_broadcast([P, NB, FE]), op=AL.mult)
                nc.vector.tensor_tensor(
                    out=hr3, in0=hr3,
                    in1=BE_s[l][:, 0:FE].unsqueeze(1)
                    .to_broadcast([P, NB, FE]), op=AL.add)
            # ELU + residual.  |y| <= sqrt(FE) so raw exp is safe (TRIV);
            # general path clamps first.
            hout = h_res[l][:, b0 * FE:b1 * FE]
            if NO_CUSTOM:
                RL = npool.tile([P, NF], BF16, tag='RL')
                nc.vector.tensor_scalar_max(RL[:], hr, 0.0)
                EXM = npool.tile([P, NF], BF16, tag='EXM')
                nc.vector.tensor_scalar_min(EXM[:], hr, 0.0)
                nc.scalar.activation(out=EXM[:], in_=EXM[:], func=ACTF.Exp)
                nc.vector.scalar_tensor_tensor(
                    out=hout, in0=EXM[:], scalar=-1.0, in1=RL[:],
                    op0=AL.add, op1=AL.add)
            else:
                EX = npool.tile([P, NF], BF16, tag='EX')
                if TRIV:
                    nc.scalar.activation(out=EX[:], in_=hr, func=ACTF.Exp)
                else:
                    MN = npool.tile([P, NF], BF16, tag='MN')
                    nc.vector.tensor_scalar_min(MN[:], hr, 0.0)
                    nc.scalar.activation(out=EX[:], in_=MN[:], func=ACTF.Exp)
                nc.vector._custom_dve(ELUSEL_OP, out=hout, in0=hr,
                                      in1=EX[:], s0=-1.0)
            if l == 1:
                nc.vector.tensor_tensor(
                    out=hout, in0=hout,
                    in1=h_res[0][:, b0 * FE:b1 * FE], op=AL.add)

        def phase_p(l, b0, b1):
            """Projections for layer l, blocks [b0, b1): xl rows -> xlo
            and ag_in[l] (then AllGather into tabs[l]), xr -> xr_res."""
            H, DO, FE, DIN = LCFG[l]
            wcols = 256 if l != 2 else 64
            for b in range(b0, b1):
                if l == 0:
                    hT = xpool.tile([IND, P], BF16, tag='xo')
                    nc.scalar.dma_start(out=hT[:],
                                        in_=xTo_d[:, b * P:(b + 1) * P])
                else:
                    psT = ps2pool.tile([P, P], BF16, tag='psT')
                    nc.tensor.transpose(
                        out=psT[:],
                        in_=h_res[l - 1][:, b * 128:(b + 1) * 128],
                        identity=ident[:])
                    hT = stpool.tile([P, P], BF16, tag='hT')
                    nc.scalar.copy(out=hT[:], in_=psT[:])
                ps = pspool.tile([P, 256], F32, tag='psA')
                nc.tensor.matmul(out=ps[:, 0:wcols], lhsT=hT[:],
                                 rhs=W01_s[l][:], start=True, stop=True)
                # xl -> xlo columns (table row content; for l==2 only the
                # first 32 cols are meaningful, the rest is stale garbage
                # that no consumer reads)
                xlo_sl = xlo[:, b * 128:b * 128 + FE]
                if TRIV:
                    nc.vector.tensor_copy(out=xlo_sl, in_=ps[:, 0:FE])
                    nc.vector.tensor_copy(
                        out=xr_res[:, b * FE:(b + 1) * FE],
                        in_=ps[:, (128 if l != 2 else 32):
                               (128 if l != 2 else 32) + FE])
                else:
                    C0 = 128 if l != 2 else 32
                    nc.vector.tensor_tensor(
                        out=xlo_sl, in0=ps[:, 0:FE],
                        in1=BL01_s[l][:, 0:FE], op=AL.add)
                    nc.vector.tensor_tensor(
                        out=xr_res[:, b * FE:(b + 1) * FE],
                        in0=ps[:, C0:C0 + FE],
                        in1=BL01_s[l][:, C0:C0 + FE], op=AL.add)
                if b == NBLK - 1:
                    # pad rows: zero real content, add poison (cores 1, 6)
                    stg = stpool.tile([P, 128], BF16, tag='stg')
                    nc.vector.tensor_scalar_mul(
                        stg[:], xlo[:, b * 128:(b + 1) * 128], pmask_s[:])
                    nc.vector.tensor_tensor(out=stg[:], in0=stg[:],
                                            in1=WC_s[l][:], op=AL.add)
                    nc.sync.dma_start(out=ag_in[l][b * P:(b + 1) * P, :],
                                      in_=stg[:])
                else:
                    nc.sync.dma_start(
                        out=ag_in[l][b * P:(b + 1) * P, :],
                        in_=xlo[:, b * 128:(b + 1) * 128])

        # ---------------- the three GAT layers ----------------
        # table rows are block-interleaved with rotation ROT: the AllGather
        # writes each core-chunk through a strided view so node (c, b, p)
        # lands at row ((b+ROT)%NBLK * NC + c)*P + p.
        CH0B = st['CH0_BLK']        # == NBLK - ROT (no-wrap chunking)
        CH0 = CH0B * P

        def ag_chunk(l, c):
            # AllGather concatenates core-major into the staging buffer,
            # then one strided HBM->HBM copy interleaves it into tabs so
            # node (c, b, p) sits at row ((b+ROT)%NBLK * NC + c)*P + p.
            v = tabs[l][:].rearrange('(rb c p) f -> c rb (p f)', c=NC, p=P)
            if c == 0:
                nb, s0, st0, rb = CH0B, 0, 0, ROT
            else:
                nb, s0, st0, rb = ROT, CH0, NC * CH0, 0
            nc.gpsimd.collective_compute(
                'AllGather', AL.bypass, replica_groups=[list(range(NC))],
                ins=[ag_in[l][s0:s0 + nb * P, :]],
                outs=[tab_stage[l][st0:st0 + NC * nb * P, :]])
            nc.sync.dma_start(
                out=v[:, rb:rb + nb, :],
                in_=tab_stage[l][st0:st0 + NC * nb * P, :]
                .rearrange('(c b p) f -> c b (p f)', c=NC, p=P))

        phase_p(0, 0, CH0B)
        ag_chunk(0, 0)
        phase_p(0, CH0B, NBLK)
        ag_chunk(0, 1)
        for l in range(3):
            att_s = attpool.tile(
                [P, KTMAX * 128], BF16, tag='att', name=f'att{l}')
            nc.sync.dma_start(
                out=att_s[:, 0:KTMAX * (128 if l != 2 else 32)],
                in_=ATTB_d[l][:])
            for b in range(CH0B):
                edge_block(l, b, att_s)
            if l < 2:
                # chunk-0 tail/projection/AllGather overlap the remaining
                # edge blocks (which only read tabs[l] and cols >= CH0B)
                layer_tail(l, 0, CH0B)
                phase_p(l + 1, 0, CH0B)
                ag_chunk(l + 1, 0)
            for b in range(CH0B, NBLK):
                edge_block(l, b, att_s)
            if l < 2:
                layer_tail(l, CH0B, NBLK)
                phase_p(l + 1, CH0B, NBLK)
                ag_chunk(l + 1, 1)
            else:
                layer_tail(l, 0, NBLK)

        # ---------------- MLP head ----------------
        # stage 1: per 3-block group, one [3*32, P] transpose and one matmul
        # against a block-diagonal stacked cW1 [96, 48].
        C1t = hpool.tile([P, NBLK * 16], F32, tag='C1')
        for j in range(cdiv(NBLK, 3)):
            nb = min(3, NBLK - 3 * j)
            psT = ps2pool.tile([P, P], BF16, tag='psT')
            nc.tensor.transpose(
                out=psT[:nb * 32, :],
                in_=h_res[2][:, j * 96:j * 96 + nb * 32],
                identity=ident[:])
            h2T = stpool.tile([P, P], BF16, tag='h2T')
            nc.scalar.copy(out=h2T[:nb * 32, :], in_=psT[:nb * 32, :])
            ps1 = pspool.tile([P, 48], F32, tag='psM')
            nc.tensor.matmul(out=ps1[:, 0:nb * 16], lhsT=h2T[:nb * 32, :],
                             rhs=cW1_s[0:nb * 32, 0:nb * 16],
                             start=True, stop=True)
            if TRIV:
                nc.vector.tensor_copy(
                    out=C1t[:, j * 48:j * 48 + nb * 16],
                    in_=ps1[:, 0:nb * 16])
            else:
                nc.vector.tensor_tensor(
                    out=C1t[:, j * 48:j * 48 + nb * 16],
                    in0=ps1[:, 0:nb * 16], in1=cb1_s[:, 0:nb * 16],
                    op=AL.add)
        EX1 = npool.tile([P, NBLK * 16], BF16, tag='EX1')
        E1 = npool.tile([P, NBLK * 16], BF16, tag='E1')
        if NO_CUSTOM:
            RL1 = npool.tile([P, NBLK * 16], BF16, tag='RL1')
            nc.vector.tensor_scalar_max(RL1[:], C1t[:], 0.0)
            EXM1 = npool.tile([P, NBLK * 16], BF16, tag='EXM1')
            nc.vector.tensor_scalar_min(EXM1[:], C1t[:], 0.0)
            nc.scalar.activation(out=EXM1[:], in_=EXM1[:], func=ACTF.Exp)
            nc.vector.scalar_tensor_tensor(
                out=E1[:], in0=EXM1[:], scalar=-1.0, in1=RL1[:],
                op0=AL.add, op1=AL.add)
        else:
            nc.scalar.activation(out=EX1[:], in_=C1t[:], func=ACTF.Exp)
            nc.vector._custom_dve(ELUSEL_OP, out=E1[:], in0=C1t[:],
                                  in1=EX1[:], s0=-1.0)
        # stage 2: per 6-block group, one [6*16, P] transpose and one matmul
        # against a block-diagonal stacked cW2 [96, 6].
        for j in range(cdiv(NBLK, 6)):
            nb = min(6, NBLK - 6 * j)
            psT = ps2pool.tile([P, P], BF16, tag='psT')
            nc.tensor.transpose(out=psT[:nb * 16, :],
                                in_=E1[:, j * 96:j * 96 + nb * 16],
                                identity=ident[:])
            e1T = stpool.tile([P, P], BF16, tag='e1T')
            nc.scalar.copy(out=e1T[:nb * 16, :], in_=psT[:nb * 16, :])
            ps2 = pspool.tile([P, 48], F32, tag='psM')
            nc.tensor.matmul(out=ps2[:, 0:nb],
                             lhsT=e1T[:nb * 16, :],
                             rhs=cW2_s[0:nb * 16, 0:nb],
                             start=True, stop=True)
            if TRIV:
                nc.vector.tensor_copy(out=out_sb[:, j * 6:j * 6 + nb],
                                      in_=ps2[:, 0:nb])
            else:
                nc.vector.tensor_tensor(out=out_sb[:, j * 6:j * 6 + nb],
                                        in0=ps2[:, 0:nb],
                                        in1=cb2_s[:].to_broadcast([P, nb]),
                                        op=AL.add)
        nc.sync.dma_start(out=out_d[:].rearrange('(b p) -> p b', p=P),
                          in_=out_sb[:])

    nc.compile()
    return nc


# ----------------------------------------------------------------------------
# entry point
# ----------------------------------------------------------------------------

def _poison_row(att_eff, fe):
    """xl row w s.t. logits att_eff . T'(w + xr) ~ -1e5 for every head,
    with T'(z) = 3z + 2|z|."""
    A = att_eff.reshape(-1, fe).astype(np.float64)
    t = np.linalg.lstsq(A, -np.ones(A.shape[0]), rcond=None)[0]
    t = t / np.abs(t).max() if np.abs(t).max() > 0 else t
    # rescale so logits ~ -1e5 (dominates |xr| perturbations ~ 1e2)
    t = t * (1e5 / max(1e-9, float(np.abs(A @ t).min())))
    w = np.where(t >= 0, t / 5.0, t)
    return w.astype(np.float32)


def _make_in_maps(st, inputs, xT_own, idx_all, pm):
    Wl = [np.asarray(inputs[f'Wl{l}'], np.float32) for l in range(3)]
    Wr = [np.asarray(inputs[f'Wr{l}'], np.float32) for l in range(3)]
    KT = [st['K_A'][b] + st['K_B'][b] for b in range(len(st['K_A']))]
    KTMAX = max(KT) + 1                      # +1 local self slot
    import ml_dtypes
    att_sc = (1.0 + NEG_SLOPE) / 2.0 / 3.0
    attb = []
    wrow = []
    NSH_REAL, NBLK = st['NSH_REAL'], len(st['K_A'])
    PAD_P0 = NSH_REAL - (NBLK - 1) * P
    for l, fe in ((0, 128), (1, 128), (2, 32)):
        a = (np.asarray(inputs[f'att{l}'], np.float32).reshape(-1) * att_sc)
        attb.append(np.ascontiguousarray(
            np.tile(a, (P, KTMAX)).astype(ml_dtypes.bfloat16)))
        w = _poison_row(a, fe)
        wc = np.zeros((P, 128), np.float32)
        wc[PAD_P0:, 0:fe] = w
        wrow.append(wc.astype(ml_dtypes.bfloat16))
    shared = {
        'xT_own': None,
        'W01_0': np.ascontiguousarray(np.concatenate([Wl[0], Wr[0]], 1)).astype(ml_dtypes.bfloat16),
        'W01_1': np.ascontiguousarray(np.concatenate([Wl[1], Wr[1]], 1)).astype(ml_dtypes.bfloat16),
        'W01_2': np.ascontiguousarray(np.concatenate([Wl[2], Wr[2]], 1)).astype(ml_dtypes.bfloat16),
        'attb_0': attb[0], 'attb_1': attb[1], 'attb_2': attb[2],
        'cW1': _blkdiag(np.asarray(inputs['cW1'], np.float32), 3),
        'cW2': _blkdiag(np.asarray(inputs['cW2'], np.float32), 6),
        'ident': np.eye(P, dtype=np.float32).astype(ml_dtypes.bfloat16),
    }
    if not st['trivial']:
        shared.update({
            'bl01_0': _rep(np.concatenate([inputs['bl0'], inputs['br0']])),
            'bl01_1': _rep(np.concatenate([inputs['bl1'], inputs['br1']])),
            'bl01_2': _rep(np.concatenate([inputs['bl2'], inputs['br2']])),
            'g_0': _rep(inputs['g0']), 'g_1': _rep(inputs['g1']),
            'g_2': _rep(inputs['g2']),
            'be_0': _rep(inputs['be0']), 'be_1': _rep(inputs['be1']),
            'be_2': _rep(inputs['be2']),
            'bo_0': _rep(inputs['bo0']), 'bo_1': _rep(inputs['bo1']),
            'bo_2': _rep(inputs['bo2']),
            'cb1': _rep(np.tile(np.asarray(inputs['cb1'], np.float32), 3)),
            'cb2': _rep(inputs['cb2']),
        })
    in_maps = []
    zero_wc = np.zeros((P, 128), ml_dtypes.bfloat16)
    for c in range(NC):
        m = dict(shared)
        m['padmask'] = pm
        m['xT_own'] = xT_own[c].astype(ml_dtypes.bfloat16)
        m['idx_all'] = idx_all[c]
        for l in range(3):
            m[f'wc_{l}'] = wrow[l] if c in (1, 6) else zero_wc
        in_maps.append(m)
    return in_maps


def _check_trivial(inputs):
    zs = ['bl0', 'br0', 'bl1', 'br1', 'bl2', 'br2', 'bo0', 'bo1', 'bo2',
          'be0', 'be1', 'be2', 'cb1', 'cb2']
    on = ['g0', 'g1', 'g2']
    for k in zs:
        if not np.all(np.asarray(inputs[k]) == 0.0):
            return False
    for k in on:
        if not np.all(np.asarray(inputs[k]) == 1.0):
            return False
    return True


_CACHE = {}
_PREP_CACHE = {}


def _run_sim(nc, in_maps):
    from concourse.bass_interp import MultiCoreSim
    sim = MultiCoreSim(nc, num_cores=NC, trace=False,
                       require_finite=False, require_nnan=False)
    cores = list(sim.cores.values())
    for c in range(NC):
        for k, v in in_maps[c].items():
            cores[c].tensor(k)[:] = v
    sim.simulate(check_with_hw=False)
    return [{'out': np.array(cores[c].tensor('out'))} for c in range(NC)]


def kernel(trace=False, backend='hw', **inputs):
    import hashlib
    x = np.asarray(inputs['x'], np.float32)
    ei = np.asarray(inputs['edge_index'])
    pkey = (x.shape, ei.shape,
            hashlib.sha1(np.ascontiguousarray(x)).hexdigest(),
            hashlib.sha1(np.ascontiguousarray(ei)).hexdigest())
    if pkey not in _PREP_CACHE:
        _PREP_CACHE.clear()
        _PREP_CACHE[pkey] = _prep(x, ei)
    st, xT_own, idx_all, row, padmask = _PREP_CACHE[pkey]
    import os as _os
    st = dict(st)
    st['trivial'] = _check_trivial(inputs)
    st['gmax'] = _os.environ.get('GMAX', '7')
    st['scratch'] = _os.environ.get('SCRATCH', '16384')
    st['fuse2'] = _os.environ.get('FUSE2', '1')
    st['nocustom'] = _os.environ.get('NO_CUSTOM', '0')
    st['altw'] = _os.environ.get('ALTW', '1')
    skey = str(sorted((k, str(v)) for k, v in st.items()))
    if skey not in _CACHE:
        _CACHE[skey] = _build(st)
    nc = _CACHE[skey]
    in_maps = _make_in_maps(st, inputs, xT_own, idx_all, padmask)
    if backend == 'sim':
        results = _run_sim(nc, in_maps)
        res = None
    else:
        res = bass_utils.run_bass_kernel_spmd(
            nc, in_maps, core_ids=list(range(NC)), trace=trace)
        results = res.results
    cat = np.concatenate([results[c]['out'] for c in range(NC)])
    out = cat[row]
    if trace:
        kernel.last_results = res
    return out.astype(np.float32)



# revision 18
# speedup vs baseline: 1.0714x; 1.0714x over previous
"""Self-contained Trainium2 Bass kernel for nn_DualGATv2 (3-layer GATv2 + MLP).

V3. Sharding: nodes dealt across 8 NeuronCores into 49 blocks of 8x128,
grouped by (in-degree, forced-window-count) through a 6-round fixed-point
iteration so each block's padded slot count is near the max-degree lower
bound; weights replicated. Per-layer projected source features live in a
bf16 HBM table built via two strided AllGathers (block-interleaved row
layout with rotation so the pad/poison block is reachable from both int16
index windows). Per-edge messages are fetched with whole-block gpsimd
dma_gather calls; self-loops never touch DMA - each block's last slot is
filled from the locally-projected xl via a vector copy.

Edge math per 128-dst block (dst-major, k = padded incoming-edge slots):
  fused custom DVE op computes t = 3z + 2|z| (z = msg + xr) equal to
  leakyrelu(z)*6/(1+s) with the constant folded into att; att-mult at DVE
  2x; logits reduce per (k,head); scatter-softmax is exp() without
  max-subtraction (logits bounded, pad slots gather poison rows that force
  logits ~ -1e5); alpha-weighted aggregation in contiguous (k,h,d) layout.
  Per-layer tails (1/denominator, LayerNorm, ELU, residual) are batched
  across blocks, and chunk-0 tails/projections/AllGather are issued mid-
  layer so they overlap the remaining edge blocks' gathers.
"""
import sys
import numpy as np

sys.path.insert(0, '/opt/trn_rl_repo')

import concourse.bass as bass
import concourse.bacc as bacc
import concourse.tile as tile
from concourse import mybir, library_config
from concourse import bass_utils
from concourse._compat import cdiv

F32 = mybir.dt.float32
BF16 = mybir.dt.bfloat16
I16 = mybir.dt.int16
AL = mybir.AluOpType
ACTF = mybir.ActivationFunctionType
AX = mybir.AxisListType

NC = 8
P = 128
HID = 32
HEADS = 4
NEG_SLOPE = 0.2
LN_EPS = 1e-5
NEG_BIG = -1.0e30
IDX_WIN = 32768

# ----------------------------------------------------------------------------
# custom DVE ops (registered through the documented dve_ops extension point)
# ----------------------------------------------------------------------------

_DVE_REGISTERED = {}


def _register_dve_ops():
    if _DVE_REGISTERED:
        return _DVE_REGISTERED
    from concourse import dve_ops
    from concourse.dve_spec import (Spec, Src0, Src1, C0, Zero, maxx, select,
                                    lower)
    from concourse.dve_uop import DveOpSpec

    def _mk(name, spec):
        for op in dve_ops.OPS:
            if op.name == name:
                _DVE_REGISTERED[name] = op
                return op
        shas = {}
        for ver in ('v3', 'v4'):
            tmp = DveOpSpec(name=name, opcode=1, uops=lower(spec, ver=ver))
            shas[ver] = tmp.sha(ver)
        op = dve_ops.DveOp(name, spec, subdim=False, uops_sha=shas)
        dve_ops.OPS.append(op)
        dve_ops.CUSTOM_DVE_SPECS[name] = spec
        dve_ops._SUB_OPCODE_FOR_NAME[name] = (
            dve_ops._CUSTOM_DVE_ROW_BASE + len(dve_ops.OPS) - 1)
        assert dve_ops._SUB_OPCODE_FOR_NAME[name] < 0x20
        _DVE_REGISTERED[name] = op
        return op

    # t = 3*z + 2*|z| (single stream, no scalar slots).  For slope 0.2 this
    # equals leakyrelu(z) * 2/(1+s) * 3, folded into the att constant.
    def _lrelu_ref(in0, in1, s0, s1, imm2):
        z = in0.astype(np.float32)
        return 3.0 * z + 2.0 * np.abs(z)

    _z = Src0
    _a = maxx(_z, Zero - _z)
    _mk('GAT_LRELU_ANT', Spec(
        body=((_z + _z) + _z) + (_a + _a),
        reference=_lrelu_ref,
    ))
    # two-stream variant: z = in0 + in1 computed inline (no scalar slots)
    def _lrelu2_ref(in0, in1, s0, s1, imm2):
        z = in0.astype(np.float32) + \
            np.asarray(in1, np.float32).reshape(in0.shape)
        return 3.0 * z + 2.0 * np.abs(z)

    _z2 = Src0 + Src1
    _a2 = maxx(_z2, Zero - _z2)
    _mk('GAT_LRELU2_ANT', Spec(
        body=((_z2 + _z2) + _z2) + (_a2 + _a2),
        reference=_lrelu2_ref,
    ))
    # elu select: out = in0 if in0 > 0 else in1 + c0   (in1 = exp(in0))
    def _elusel_ref(in0, in1, s0, s1, imm2):
        i1 = np.asarray(in1, np.float32).reshape(in0.shape)
        return np.where(in0 > 0, in0.astype(np.float32), i1 + s0)

    _mk('GAT_ELUSEL_ANT', Spec(
        body=select(Src0 > Zero, Src0, Src1 + C0),
        reference=_elusel_ref,
    ))
    return _DVE_REGISTERED


# ----------------------------------------------------------------------------
# host-side preprocessing
# ----------------------------------------------------------------------------

def _prep(x, edge_index):
    x = np.asarray(x, dtype=np.float32)
    N = x.shape[0]
    # self-loops are handled locally in the kernel (slot kt of each block is
    # filled from the locally-projected xl), so only real edges are gathered.
    src = np.asarray(edge_index[0], dtype=np.int64)
    dst = np.asarray(edge_index[1], dtype=np.int64)

    deg = np.bincount(dst, minlength=N)       # in-degree, no self loop

    NSH_REAL = cdiv(N, NC)
    NSH = cdiv(NSH_REAL + 1, P) * P       # >= 1 pad row per core
    NBLK = NSH // P
    TAB = NC * NSH
    # Table rows are block-interleaved with rotation ROT: node at
    # (core, pos=b*128+p) lives at table row ((b+ROT)%NBLK * NC + core)*P+p.
    # Built by TWO strided AllGather chunks with no wraparound:
    #   chunk0 = blocks [0, NBLK-ROT) -> row-blocks [ROT, NBLK)
    #   chunk1 = blocks [NBLK-ROT, NBLK) -> row-blocks [0, ROT)
    # ROT=20 puts the pad block (NBLK-1) at row-block 19, inside the flex
    # region [HI_BASE, W_LO) so both windows can reach the poison rows.
    ROT = 20
    CH0_BLK = NBLK - ROT
    W_LO = min(IDX_WIN, TAB)
    HI_BASE = max(0, TAB - IDX_WIN)

    def place(order):
        ranks = np.arange(N)
        g, j = ranks // NC, ranks % NC
        core_of_rank = np.where(g % 2 == 0, j, NC - 1 - j)
        core = np.zeros(N, dtype=np.int64)
        core[order] = core_of_rank
        pos = np.zeros(N, dtype=np.int64)
        pos[order] = g
        return core, pos

    def rows_of(core, pos):
        b = pos // P
        p = pos % P
        rb = (b + ROT) % NBLK
        return (rb * NC + core) * P + p

    # Iterated placement: regroup nodes by (in-degree, forced-A count) so
    # each 1024-node block is homogeneous in both -> minimal slot padding.
    order = np.argsort(-deg, kind='stable')
    for it in range(6):
        core, pos = place(order)
        row = rows_of(core, pos)
        rs = row[src]
        fA = np.bincount(dst[rs < HI_BASE], minlength=N)   # forced A
        fB = np.bincount(dst[rs >= W_LO], minlength=N)     # forced B
        if it < 5:
            order = np.lexsort((-fA, -deg))

    pad_p0 = NSH_REAL - (NBLK - 1) * P
    PZ_LO = int(rows_of(np.int64(1), np.int64(NSH_REAL)))
    PZ_HI = int(rows_of(np.int64(6), np.int64(NSH_REAL))) - HI_BASE
    assert 0 <= PZ_LO < W_LO and 0 <= PZ_HI < IDX_WIN

    e_order = np.argsort(dst, kind='stable')
    src_s = src[e_order]
    dst_s = dst[e_order]
    starts = np.searchsorted(dst_s, np.arange(N))
    ends = np.searchsorted(dst_s, np.arange(N) + 1)
    rs_all = row[src_s]

    node_at = np.full((NC, NSH), -1, dtype=np.int64)
    node_at[core, pos] = np.arange(N)

    # exact per-block split: ka = max forced-A, kb = max(max forced-B,
    # maxd - ka); per-node A-count ca = min(ka, d - forced-B).
    K_A = np.zeros(NBLK, dtype=np.int64)
    K_B = np.zeros(NBLK, dtype=np.int64)
    ca_of = np.zeros(N, dtype=np.int64)
    for b in range(NBLK):
        sl = node_at[:, b * P:(b + 1) * P].reshape(-1)
        sl = sl[sl >= 0]
        if not len(sl):
            K_A[b] = K_B[b] = 0
            continue
        d = deg[sl]
        ka = int(fA[sl].max())
        kb = max(int(fB[sl].max()), int(d.max()) - ka)
        K_A[b] = ka
        K_B[b] = kb
        ca_of[sl] = np.minimum(ka, d - fB[sl])

    SUMKT = int((K_A + K_B).sum())
    IDXW = int(8 * SUMKT)
    idx_all = np.zeros((NC, P, IDXW), dtype=np.int16)

    def wrap(flat):
        n = len(flat)
        S = cdiv(n, 16)
        a = np.zeros(16 * S, np.int16)
        a[:n] = flat
        return np.tile(a.reshape(S, 16).T, (8, 1))

    icol = 0
    for b in range(NBLK):
        ka, kb = int(K_A[b]), int(K_B[b])
        for c in range(NC):
            flatA = np.full(ka * P, PZ_LO, np.int64)
            flatB = np.full(kb * P, PZ_HI, np.int64)
            for p in range(P):
                n = node_at[c, b * P + p]
                if n < 0:
                    continue  # pad partition: denom comes from the self slot
                rs = rs_all[starts[n]:ends[n]]
                inA = rs < HI_BASE
                flex = ~inA & (rs < W_LO)
                ca = int(ca_of[n])
                nfa = int(inA.sum())
                selA = inA.copy()
                if ca > nfa:
                    fidx = np.nonzero(flex)[0]
                    selA[fidx[:ca - nfa]] = True
                ea = rs[selA]
                eb = rs[~selA] - HI_BASE
                la, lb = len(ea), len(eb)
                if la:
                    flatA[np.arange(la) * P + p] = ea
                if lb:
                    flatB[np.arange(lb) * P + p] = eb
            if ka:
                idx_all[c, :, icol:icol + 8 * ka] = \
                    wrap(flatA.astype(np.int16))
            if kb:
                idx_all[c, :, icol + 8 * ka:icol + 8 * (ka + kb)] = \
                    wrap(flatB.astype(np.int16))
        icol += 8 * (ka + kb)

    IND = x.shape[1]
    xT_own = np.zeros((NC, IND, NSH), dtype=np.float32)
    xT_own[core, :, pos] = x                            # [NC, IND, NSH]

    padmask = (np.arange(P) < pad_p0).astype(np.float32).reshape(P, 1)
    orow = core * NSH + pos                  # output unshard (pos-major)
    st = dict(N=N, NSH=NSH, NSH_REAL=NSH_REAL, NBLK=NBLK, TAB=TAB,
              W_LO=W_LO, HI_BASE=HI_BASE, K_A=K_A.tolist(),
              K_B=K_B.tolist(), SUMKT=SUMKT, IDXW=IDXW, IN_DIM=IND,
              CH0_BLK=CH0_BLK, ROT=ROT)
    return st, xT_own, idx_all, orow, padmask


def _blkdiag(w, n):
    import ml_dtypes
    r, c = w.shape
    out = np.zeros((r * n, c * n), np.float32)
    for i in range(n):
        out[i * r:(i + 1) * r, i * c:(i + 1) * c] = w
    return np.ascontiguousarray(out).astype(ml_dtypes.bfloat16)


def _rep(v):
    v = np.asarray(v, dtype=np.float32).reshape(1, -1)
    return np.ascontiguousarray(np.tile(v, (P, 1)))


# ----------------------------------------------------------------------------
# kernel builder
# ----------------------------------------------------------------------------

def _build(st):
    import os
    GMAX = int(os.environ.get('GMAX', '7'))      # ring caps a call at 57 descs/DMA
    NO_CUSTOM = os.environ.get('NO_CUSTOM', '0') == '1'
    ALTW = os.environ.get('ALTW', '1') == '1'
    _register_dve_ops()
    from concourse.dve_ops import OPS as _OPS
    LRELU_OP = next(o for o in _OPS if o.name == 'GAT_LRELU_ANT')
    LRELU2_OP = next(o for o in _OPS if o.name == 'GAT_LRELU2_ANT')
    FUSE2 = os.environ.get('FUSE2', '1') == '1'
    ELUSEL_OP = next(o for o in _OPS if o.name == 'GAT_ELUSEL_ANT')

    NSH, NBLK, TAB = st['NSH'], st['NBLK'], st['TAB']
    NSH_REAL = st['NSH_REAL']
    W_LO, HI_BASE = st['W_LO'], st['HI_BASE']
    K_A, K_B = st['K_A'], st['K_B']
    SUMKT = st['SUMKT']
    IDXW = st['IDXW']
    IND = st['IN_DIM']
    ROT = st['ROT']
    KT = [K_A[b] + K_B[b] for b in range(NBLK)]     # gathered edge slots
    KTMAX = max(KT) + 1                             # +1 local self slot
    TRIV = st['trivial']  # biases zero, gamma ones => skip those ops

    # layer cfg: (heads, dout, feat, in_feat)
    LCFG = [(HEADS, HID, HEADS * HID, IND),
            (HEADS, HID, HEADS * HID, HEADS * HID),
            (1, HID, HID, HEADS * HID)]

    SCRATCH = int(os.environ.get('SCRATCH', '16384'))
    nc = bacc.Bacc('TRN2', target_bir_lowering=False, debug=False,
                   enable_asserts=True, num_devices=NC,
                   num_swdge_queues=4,
                   dynamic_dma_scratch_size=SCRATCH)

    def ein(name, shape, dt=F32):
        return nc.dram_tensor(name, shape, dt, kind='ExternalInput')

    xTo_d = ein('xT_own', [IND, NSH], BF16)
    idx_d = ein('idx_all', [P, IDXW], I16)
    pmask_d = ein('padmask', [P, 1])
    WC_d = [ein(f'wc_{l}', [P, 128], BF16) for l in range(3)]
    W01_d = [ein('W01_0', [IND, 256], BF16),
             ein('W01_1', [128, 256], BF16),
             ein('W01_2', [128, 64], BF16)]
    ATTB_d = [ein('attb_0', [P, KTMAX * 128], BF16),
              ein('attb_1', [P, KTMAX * 128], BF16),
              ein('attb_2', [P, KTMAX * 32], BF16)]
    cW1_d = ein('cW1', [96, 48], BF16)
    cW2_d = ein('cW2', [96, 6], BF16)
    ident_d = ein('ident', [P, P], BF16)
    if not TRIV:
        BL01_d = [ein('bl01_0', [P, 256]), ein('bl01_1', [P, 256]),
                  ein('bl01_2', [P, 64])]
        GG_d = [ein('g_0', [P, 128]), ein('g_1', [P, 128]),
                ein('g_2', [P, 32])]
        BE_d = [ein('be_0', [P, 128]), ein('be_1', [P, 128]),
                ein('be_2', [P, 32])]
        BO_d = [ein('bo_0', [P, 128]), ein('bo_1', [P, 128]),
                ein('bo_2', [P, 32])]
        cb1_d = ein('cb1', [P, 48])
        cb2_d = ein('cb2', [P, 1])
    out_d = nc.dram_tensor('out', [NSH], F32, kind='ExternalOutput')

    tabs = [nc.dram_tensor(f'table{l}', [TAB, 128], BF16, kind='Internal')
            for l in range(3)]
    tab_stage = [nc.dram_tensor(f'tstage{l}', [TAB, 128], BF16,
                                kind='Internal', addr_space='Shared')
                 for l in range(3)]
    ag_in = [nc.dram_tensor(f'ag_in{l}', [NSH, 128], BF16, kind='Internal')
             for l in range(3)]

    import contextlib
    with tile.TileContext(nc) as tc, contextlib.ExitStack() as ctx:
        cpool = ctx.enter_context(tc.tile_pool(name='consts', bufs=1))
        attpool = ctx.enter_context(tc.tile_pool(name='att', bufs=1))
        gpool = ctx.enter_context(tc.tile_pool(name='g', bufs=3))
        tpool = ctx.enter_context(tc.tile_pool(name='t', bufs=2))
        spool = ctx.enter_context(tc.tile_pool(name='small', bufs=3))
        npool = ctx.enter_context(tc.tile_pool(name='node', bufs=1))
        hpool = ctx.enter_context(tc.tile_pool(name='h', bufs=1))
        xpool = ctx.enter_context(tc.tile_pool(name='xt', bufs=4))
        stpool = ctx.enter_context(tc.tile_pool(name='stage', bufs=4))
        pspool = ctx.enter_context(tc.tile_pool(name='ps', bufs=3,
                                                space='PSUM'))
        ps2pool = ctx.enter_context(tc.tile_pool(name='ps2', bufs=2,
                                                 space='PSUM'))

        def load_const(pool, dram, shape, dt=F32):
            t = pool.tile(shape, dt, tag='c_' + dram.name,
                          name='c_' + dram.name)
            nc.sync.dma_start(out=t[:], in_=dram[:])
            return t

        ident = load_const(cpool, ident_d, [P, P], BF16)
        W01_s = [load_const(cpool, W01_d[l], list(W01_d[l].shape), BF16)
                 for l in range(3)]
        cW1_s = load_const(cpool, cW1_d, [96, 48], BF16)
        cW2_s = load_const(cpool, cW2_d, [96, 6], BF16)
        if not TRIV:
            BL01_s = [load_const(cpool, BL01_d[l], list(BL01_d[l].shape))
                      for l in range(3)]
            GG_s = [load_const(cpool, GG_d[l], list(GG_d[l].shape))
                    for l in range(3)]
            BE_s = [load_const(cpool, BE_d[l], list(BE_d[l].shape))
                    for l in range(3)]
            BO_s = [load_const(cpool, BO_d[l], list(BO_d[l].shape))
                    for l in range(3)]
            cb1_s = load_const(cpool, cb1_d, [P, 48])
            cb2_s = load_const(cpool, cb2_d, [P, 1])

        pmask_s = load_const(cpool, pmask_d, [P, 1])
        eps_t = cpool.tile([P, 1], F32, tag='eps', name='eps')
        nc.vector.memset(eps_t[:], float(LN_EPS))
        idx_s = cpool.tile([P, IDXW], I16, tag='idx')
        nc.sync.dma_start(out=idx_s[:], in_=idx_d[:])
        WC_s = [load_const(cpool, WC_d[l], [P, 128], BF16)
                for l in range(3)]

        h_res = [hpool.tile([P, NBLK * 128], BF16, tag='h0', name='h0'),
                 hpool.tile([P, NBLK * 128], BF16, tag='h1', name='h1'),
                 hpool.tile([P, NBLK * HID], BF16, tag='h2', name='h2')]
        h_raw = hpool.tile([P, NBLK * 128], F32, tag='hraw')
        xr_res = hpool.tile([P, NBLK * 128], BF16, tag='xr')
        xlo = hpool.tile([P, NBLK * 128], BF16, tag='xlo', name='xlo')
        DN_all = hpool.tile([P, NBLK * HEADS], F32, tag='dn')
        out_sb = hpool.tile([P, NBLK], F32, tag='outsb')

        # per-block idx/mask column offsets
        ic_of, mc_of = [], []
        ic = mc = 0
        for b in range(NBLK):
            ic_of.append(ic)
            mc_of.append(mc)
            ic += 8 * KT[b]
            mc += KT[b]

        qc = [0]

        def edge_block(l, b, att_s):
            H, DO, FE, _ = LCFG[l]
            ka, kb = K_A[b], K_B[b]
            kt = ka + kb          # gathered edge slots
            kt1 = kt + 1          # + local self-loop slot
            G = gpool.tile([P, kt1, 128], BF16, tag='G')
            gm_a = GMAX if GMAX else max(ka, 1)
            gm_b = GMAX if GMAX else max(kb, 1)
            for off in range(0, ka, gm_a):
                kk = min(gm_a, ka - off)
                nc.gpsimd.dma_gather(
                    G[:, off:off + kk, :], tabs[l][0:W_LO, :],
                    idx_s[:, ic_of[b] + 8 * off:ic_of[b] + 8 * (off + kk)],
                    kk * P, kk * P, 128, queue_num=qc[0] % 4)
                qc[0] += 1
            for off in range(0, kb, gm_b):
                kk = min(gm_b, kb - off)
                nc.gpsimd.dma_gather(
                    G[:, ka + off:ka + off + kk, :], tabs[l][HI_BASE:TAB, :],
                    idx_s[:, ic_of[b] + 8 * (ka + off):
                          ic_of[b] + 8 * (ka + off + kk)],
                    kk * P, kk * P, 128, queue_num=qc[0] % 4)
                qc[0] += 1
            # self-loop slot: own projected xl, no gather
            nc.vector.tensor_copy(out=G[:, kt, 0:FE],
                                  in_=xlo[:, b * 128:b * 128 + FE])
            xr_col = xr_res[:, b * FE:(b + 1) * FE]
            T = tpool.tile([P, kt1 * FE], BF16, tag='T')
            T2 = tpool.tile([P, kt1 * FE], BF16, tag='T2')
            T3 = T[:].rearrange('p (k f) -> p k f', k=kt1)
            G3 = G[:, :, 0:FE]
            # t = 3z + 2|z|, z = msg + xr (leaky; 1/3*(1+s)/2 inside att)
            if NO_CUSTOM:
                nc.vector.tensor_tensor(
                    out=T3, in0=G3,
                    in1=xr_col.unsqueeze(1).to_broadcast([P, kt1, FE]),
                    op=AL.add)
                nc.vector.tensor_scalar_mul(
                    T[:], T[:], float(6.0 / (1.0 + NEG_SLOPE)))
                nc.vector.scalar_tensor_tensor(
                    out=T[:], in0=T[:], scalar=float(NEG_SLOPE), in1=T[:],
                    op0=AL.mult, op1=AL.max)
            elif FUSE2:
                nc.vector._custom_dve(
                    LRELU2_OP, out=T3, in0=G3,
                    in1=xr_col.unsqueeze(1).to_broadcast([P, kt1, FE]))
            else:
                nc.vector.tensor_tensor(
                    out=T3, in0=G3,
                    in1=xr_col.unsqueeze(1).to_broadcast([P, kt1, FE]),
                    op=AL.add)
                nc.vector._custom_dve(LRELU_OP, out=T[:], in0=T[:])
            # *= att' (pre-tiled; out-of-place so the 2x bf16 uop applies)
            nc.vector.tensor_tensor(out=T2[:], in0=T[:],
                                    in1=att_s[:, 0:kt1 * FE], op=AL.mult)
            # logits: sum over d per (k,h)
            LG = spool.tile([P, kt1 * H], F32, tag='LG')
            nc.vector.tensor_reduce(
                out=LG[:], in_=T2[:].rearrange('p (kh d) -> p kh d', d=DO),
                axis=AX.X, op=AL.add)
            # softmax numerator (no max subtraction; logits are bounded,
            # pad slots carry poison-row logits of about -1e5)
            A = spool.tile([P, kt1 * H], BF16, tag='A')
            if H == 1:
                nc.scalar.activation(out=A[:], in_=LG[:], func=ACTF.Exp,
                                     accum_out=DN_all[:, b:b + 1])
            else:
                nc.scalar.activation(out=A[:], in_=LG[:], func=ACTF.Exp)
                nc.vector.tensor_reduce(
                    out=DN_all[:, b * H:(b + 1) * H],
                    in_=A[:].rearrange('p (k h) -> p h k', h=H),
                    axis=AX.X, op=AL.add)
            if ALTW:
                # weighted messages in native (k, h, d) layout: mult is
                # fully contiguous (2x bf16); the k-sum is a halving
                # fold-tree of contiguous bf16 adds ping-ponging T/T2
                # (every op out-of-place so the 2x uop applies).
                W = T[:].rearrange('p (k h d) -> p k h d', h=H, d=DO)
                Gv = G[:, :, 0:FE].rearrange('p k (h d) -> p k h d', h=H)
                Av = A[:].rearrange('p (k h) -> p k h', h=H) \
                    .unsqueeze(3).to_broadcast([P, kt1, H, DO])
                nc.vector.tensor_tensor(out=W, in0=Gv, in1=Av, op=AL.mult)
                k = kt1
                cur, oth = T, T2
                while k > 2:
                    h2 = k // 2
                    cv = cur[:].rearrange('p (k f) -> p k f', f=FE)
                    ov = oth[:].rearrange('p (k f) -> p k f', f=FE)
                    nc.vector.tensor_tensor(
                        out=ov[:, 0:h2, :], in0=cv[:, 0:h2, :],
                        in1=cv[:, k - h2:k, :], op=AL.add)
                    if k & 1:
                        nc.vector.tensor_copy(out=ov[:, h2, :],
                                              in_=cv[:, h2, :])
                    k = k - h2
                    cur, oth = oth, cur
                cv = cur[:].rearrange('p (k f) -> p k f', f=FE)
                if k == 2:
                    nc.vector.tensor_tensor(
                        out=h_raw[:, b * FE:(b + 1) * FE],
                        in0=cv[:, 0, :], in1=cv[:, 1, :], op=AL.add)
                else:
                    nc.vector.tensor_copy(
                        out=h_raw[:, b * FE:(b + 1) * FE], in_=cv[:, 0, :])
            else:
                # weighted messages, (h d k) layout, then sum over k
                if H == 1:
                    W = T[:].rearrange('p (d k) -> p d k', k=kt1)
                    Gv = G[:, :, 0:FE].rearrange('p k d -> p d k')
                    Av = A[:].unsqueeze(1).to_broadcast([P, FE, kt1])
                else:
                    W = T[:].rearrange('p (h d k) -> p h d k', h=H, d=DO)
                    Gv = G[:, :, 0:FE].rearrange('p k (h d) -> p h d k', h=H)
                    Av = A[:].rearrange('p (k h) -> p h k', h=H) \
                        .unsqueeze(2).to_broadcast([P, H, DO, kt1])
                nc.vector.tensor_tensor(out=W, in0=Gv, in1=Av, op=AL.mult)
                nc.vector.tensor_reduce(
                    out=h_raw[:, b * FE:(b + 1) * FE], in_=W,
                    axis=AX.X, op=AL.add)

        def layer_tail(l, b0, b1):
            """Batched over blocks [b0, b1): /denom, LayerNorm, ELU, skip."""
            H, DO, FE, _ = LCFG[l]
            NB = b1 - b0
            NF = NB * FE
            hr = h_raw[:, b0 * FE:b1 * FE]
            # 1/denominator, applied per (block, head)
            R = npool.tile([P, NB * H], F32, tag='R')
            nc.vector.reciprocal_approx_fast(
                R[:], DN_all[:, b0 * H:b1 * H])
            hr4 = hr.rearrange('p (b h d) -> p b h d', h=H, d=DO)
            hr3 = hr.rearrange('p (b f) -> p b f', f=FE)
            nc.vector.tensor_tensor(
                out=hr4, in0=hr4,
                in1=R[:].rearrange('p (b h) -> p b h', h=H)
                .unsqueeze(3).to_broadcast([P, NB, H, DO]),
                op=AL.mult)
            if not TRIV:
                nc.vector.tensor_tensor(
                    out=hr3, in0=hr3,
                    in1=BO_s[l][:, 0:FE].unsqueeze(1)
                    .to_broadcast([P, NB, FE]),
                    op=AL.add)
            # LayerNorm stats, batched per block
            MU = npool.tile([P, NB], F32, tag='MU')
            nc.vector.tensor_reduce(
                out=MU[:], in_=hr3, axis=AX.X, op=AL.add)
            nc.vector.tensor_scalar_mul(MU[:], MU[:], 1.0 / FE)
            SQ = npool.tile([P, NF], BF16, tag='EX')
            nc.vector.tensor_tensor(out=SQ[:], in0=hr, in1=hr, op=AL.mult)
            SSQ = npool.tile([P, NB], F32, tag='SSQ')
            nc.vector.tensor_reduce(
                out=SSQ[:], in_=SQ[:].rearrange('p (b f) -> p b f', f=FE),
                axis=AX.X, op=AL.add)
            MM = npool.tile([P, NB], F32, tag='MM')
            nc.vector.tensor_tensor(out=MM[:], in0=MU[:], in1=MU[:],
                                    op=AL.mult)
            VAR = npool.tile([P, NB], F32, tag='VAR')
            nc.vector.scalar_tensor_tensor(
                out=VAR[:], in0=SSQ[:], scalar=1.0 / FE, in1=MM[:],
                op0=AL.mult, op1=AL.subtract)
            SD = npool.tile([P, NB], F32, tag='SD')
            nc.scalar.activation(out=SD[:], in_=VAR[:], func=ACTF.Sqrt,
                                 bias=eps_t[:])
            IV = npool.tile([P, NB], F32, tag='IV')
            nc.vector.reciprocal_approx_fast(IV[:], SD[:])
            # normalize
            nc.vector.tensor_tensor(
                out=hr3, in0=hr3,
                in1=MU[:].unsqueeze(2).to_broadcast([P, NB, FE]),
                op=AL.subtract)
            nc.vector.tensor_tensor(
                out=hr3, in0=hr3,
                in1=IV[:].unsqueeze(2).to_broadcast([P, NB, FE]),
                op=AL.mult)
            if not TRIV:
                nc.vector.tensor_tensor(
                    out=hr3, in0=hr3,
                    in1=GG_s[l][:, 0:FE].unsqueeze(1)
                    .to_broadcast([P, NB, FE]), op=AL.mult)
                nc.vector.tensor_tensor(
                    out=hr3, in0=hr3,
                    in1=BE_s[l][:, 0:FE].unsqueeze(1)
                    .to_broadcast([P, NB, FE]), op=AL.add)
            # ELU + residual.  |y| <= sqrt(FE) so raw exp is safe (TRIV);
            # general path clamps first.
            hout = h_res[l][:, b0 * FE:b1 * FE]
            if NO_CUSTOM:
                RL = npool.tile([P, NF], BF16, tag='RL')
                nc.vector.tensor_scalar_max(RL[:], hr, 0.0)
                EXM = npool.tile([P, NF], BF16, tag='EXM')
                nc.vector.tensor_scalar_min(EXM[:], hr, 0.0)
                nc.scalar.activation(out=EXM[:], in_=EXM[:], func=ACTF.Exp)
                nc.vector.scalar_tensor_tensor(
                    out=hout, in0=EXM[:], scalar=-1.0, in1=RL[:],
                    op0=AL.add, op1=AL.add)
            else:
                EX = npool.tile([P, NF], BF16, tag='EX')
                if TRIV:
                    nc.scalar.activation(out=EX[:], in_=hr, func=ACTF.Exp)
                else:
                    MN = npool.tile([P, NF], BF16, tag='MN')
                    nc.vector.tensor_scalar_min(MN[:], hr, 0.0)
                    nc.scalar.activation(out=EX[:], in_=MN[:], func=ACTF.Exp)
                nc.vector._custom_dve(ELUSEL_OP, out=hout, in0=hr,
                                      in1=EX[:], s0=-1.0)
            if l == 1:
                nc.vector.tensor_tensor(
                    out=hout, in0=hout,
                    in1=h_res[0][:, b0 * FE:b1 * FE], op=AL.add)

        def phase_p(l, b0, b1):
            """Projections for layer l, blocks [b0, b1): xl rows -> xlo
            and ag_in[l] (then AllGather into tabs[l]), xr -> xr_res."""
            H, DO, FE, DIN = LCFG[l]
            wcols = 256 if l != 2 else 64
            for b in range(b0, b1):
                if l == 0:
                    hT = xpool.tile([IND, P], BF16, tag='xo')
                    nc.scalar.dma_start(out=hT[:],
                                        in_=xTo_d[:, b * P:(b + 1) * P])
                else:
                    psT = ps2pool.tile([P, P], BF16, tag='psT')
                    nc.tensor.transpose(
                        out=psT[:],
                        in_=h_res[l - 1][:, b * 128:(b + 1) * 128],
                        identity=ident[:])
                    hT = stpool.tile([P, P], BF16, tag='hT')
                    nc.scalar.copy(out=hT[:], in_=psT[:])
                ps = pspool.tile([P, 256], F32, tag='psA')
                nc.tensor.matmul(out=ps[:, 0:wcols], lhsT=hT[:],
                                 rhs=W01_s[l][:], start=True, stop=True)
                # xl -> xlo columns (table row content; for l==2 only the
                # first 32 cols are meaningful, the rest is stale garbage
                # that no consumer reads)
                xlo_sl = xlo[:, b * 128:b * 128 + FE]
                if TRIV:
                    nc.vector.tensor_copy(out=xlo_sl, in_=ps[:, 0:FE])
                    nc.vector.tensor_copy(
                        out=xr_res[:, b * FE:(b + 1) * FE],
                        in_=ps[:, (128 if l != 2 else 32):
                               (128 if l != 2 else 32) + FE])
                else:
                    C0 = 128 if l != 2 else 32
                    nc.vector.tensor_tensor(
                        out=xlo_sl, in0=ps[:, 0:FE],
                        in1=BL01_s[l][:, 0:FE], op=AL.add)
                    nc.vector.tensor_tensor(
                        out=xr_res[:, b * FE:(b + 1) * FE],
                        in0=ps[:, C0:C0 + FE],
                        in1=BL01_s[l][:, C0:C0 + FE], op=AL.add)
                if b == NBLK - 1:
                    # pad rows: zero real content, add poison (cores 1, 6)
                    stg = stpool.tile([P, 128], BF16, tag='stg')
                    nc.vector.tensor_scalar_mul(
                        stg[:], xlo[:, b * 128:(b + 1) * 128], pmask_s[:])
                    nc.vector.tensor_tensor(out=stg[:], in0=stg[:],
                                            in1=WC_s[l][:], op=AL.add)
                    nc.sync.dma_start(out=ag_in[l][b * P:(b + 1) * P, :],
                                      in_=stg[:])
                else:
                    nc.sync.dma_start(
                        out=ag_in[l][b * P:(b + 1) * P, :],
                        in_=xlo[:, b * 128:(b + 1) * 128])

        # ---------------- the three GAT layers ----------------
        # table rows are block-interleaved with rotation ROT: the AllGather
        # concatenates core-major into the staging buffer, then one strided
        # HBM->HBM copy interleaves it into tabs so node (c, b, p) sits at
        # row ((b+ROT)%NBLK * NC + c)*P + p.  Chunk boundaries are chosen
        # so no chunk's row-block range wraps; the small tail chunk keeps
        # the layer-boundary serial segment short.
        CH0B = st['CH0_BLK']        # == NBLK - ROT (no-wrap point)
        CHUNKS = [(0, 15), (15, CH0B), (CH0B, 45), (45, NBLK)]

        def ag_chunk(l, b0, b1):
            nb = b1 - b0
            rb = (b0 + ROT) % NBLK
            assert rb + nb <= NBLK
            v = tabs[l][:].rearrange('(rb c p) f -> c rb (p f)', c=NC, p=P)
            st0 = NC * b0 * P
            nc.gpsimd.collective_compute(
                'AllGather', AL.bypass, replica_groups=[list(range(NC))],
                ins=[ag_in[l][b0 * P:b1 * P, :]],
                outs=[tab_stage[l][st0:st0 + NC * nb * P, :]])
            nc.sync.dma_start(
                out=v[:, rb:rb + nb, :],
                in_=tab_stage[l][st0:st0 + NC * nb * P, :]
                .rearrange('(c b p) f -> c b (p f)', c=NC, p=P))

        for (b0, b1) in CHUNKS:
            phase_p(0, b0, b1)
            ag_chunk(0, b0, b1)
        for l in range(3):
            att_s = attpool.tile(
                [P, KTMAX * 128], BF16, tag='att', name=f'att{l}')
            nc.sync.dma_start(
                out=att_s[:, 0:KTMAX * (128 if l != 2 else 32)],
                in_=ATTB_d[l][:])
            for (b0, b1) in CHUNKS:
                for b in range(b0, b1):
                    edge_block(l, b, att_s)
                # this chunk's tail/projection/AllGather overlap the next
                # chunk's edge blocks (disjoint xlo/xr_res columns)
                if l < 2:
                    layer_tail(l, b0, b1)
                    phase_p(l + 1, b0, b1)
                    ag_chunk(l + 1, b0, b1)
            if l == 2:
                layer_tail(l, 0, NBLK)

        # ---------------- MLP head ----------------
        # stage 1: per 3-block group, one [3*32, P] transpose and one matmul
        # against a block-diagonal stacked cW1 [96, 48].
        C1t = hpool.tile([P, NBLK * 16], F32, tag='C1')
        for j in range(cdiv(NBLK, 3)):
            nb = min(3, NBLK - 3 * j)
            psT = ps2pool.tile([P, P], BF16, tag='psT')
            nc.tensor.transpose(
                out=psT[:nb * 32, :],
                in_=h_res[2][:, j * 96:j * 96 + nb * 32],
                identity=ident[:])
            h2T = stpool.tile([P, P], BF16, tag='h2T')
            nc.scalar.copy(out=h2T[:nb * 32, :], in_=psT[:nb * 32, :])
            ps1 = pspool.tile([P, 48], F32, tag='psM')
            nc.tensor.matmul(out=ps1[:, 0:nb * 16], lhsT=h2T[:nb * 32, :],
                             rhs=cW1_s[0:nb * 32, 0:nb * 16],
                             start=True, stop=True)
            if TRIV:
                nc.vector.tensor_copy(
                    out=C1t[:, j * 48:j * 48 + nb * 16],
                    in_=ps1[:, 0:nb * 16])
            else:
                nc.vector.tensor_tensor(
                    out=C1t[:, j * 48:j * 48 + nb * 16],
                    in0=ps1[:, 0:nb * 16], in1=cb1_s[:, 0:nb * 16],
                    op=AL.add)
        EX1 = npool.tile([P, NBLK * 16], BF16, tag='EX1')
        E1 = npool.tile([P, NBLK * 16], BF16, tag='E1')
        if NO_CUSTOM:
            RL1 = npool.tile([P, NBLK * 16], BF16, tag='RL1')
            nc.vector.tensor_scalar_max(RL1[:], C1t[:], 0.0)
            EXM1 = npool.tile([P, NBLK * 16], BF16, tag='EXM1')
            nc.vector.tensor_scalar_min(EXM1[:], C1t[:], 0.0)
            nc.scalar.activation(out=EXM1[:], in_=EXM1[:], func=ACTF.Exp)
            nc.vector.scalar_tensor_tensor(
                out=E1[:], in0=EXM1[:], scalar=-1.0, in1=RL1[:],
                op0=AL.add, op1=AL.add)
        else:
            nc.scalar.activation(out=EX1[:], in_=C1t[:], func=ACTF.Exp)
            nc.vector._custom_dve(ELUSEL_OP, out=E1[:], in0=C1t[:],
                                  in1=EX1[:], s0=-1.0)
        # stage 2: per 6-block group, one [6*16, P] transpose and one matmul
        # against a block-diagonal stacked cW2 [96, 6].
        for j in range(cdiv(NBLK, 6)):
            nb = min(6, NBLK - 6 * j)
            psT = ps2pool.tile([P, P], BF16, tag='psT')
            nc.tensor.transpose(out=psT[:nb * 16, :],
                                in_=E1[:, j * 96:j * 96 + nb * 16],
                                identity=ident[:])
            e1T = stpool.tile([P, P], BF16, tag='e1T')
            nc.scalar.copy(out=e1T[:nb * 16, :], in_=psT[:nb * 16, :])
            ps2 = pspool.tile([P, 48], F32, tag='psM')
            nc.tensor.matmul(out=ps2[:, 0:nb],
                             lhsT=e1T[:nb * 16, :],
                             rhs=cW2_s[0:nb * 16, 0:nb],
                             start=True, stop=True)
            if TRIV:
                nc.vector.tensor_copy(out=out_sb[:, j * 6:j * 6 + nb],
                                      in_=ps2[:, 0:nb])
            else:
                nc.vector.tensor_tensor(out=out_sb[:, j * 6:j * 6 + nb],
                                        in0=ps2[:, 0:nb],
                                        in1=cb2_s[:].to_broadcast([P, nb]),
                                        op=AL.add)
        nc.sync.dma_start(out=out_d[:].rearrange('(b p) -> p b', p=P),
                          in_=out_sb[:])

    nc.compile()
    return nc


# ----------------------------------------------------------------------------
# entry point
# ----------------------------------------------------------------------------

def _poison_row(att_eff, fe):
    """xl row w s.t. logits att_eff . T'(w + xr) ~ -1e5 for every head,
    with T'(z) = 3z + 2|z|."""
    A = att_eff.reshape(-1, fe).astype(np.float64)
    t = np.linalg.lstsq(A, -np.ones(A.shape[0]), rcond=None)[0]
    t = t / np.abs(t).max() if np.abs(t).max() > 0 else t
    # rescale so logits ~ -1e5 (dominates |xr| perturbations ~ 1e2)
    t = t * (1e5 / max(1e-9, float(np.abs(A @ t).min())))
    w = np.where(t >= 0, t / 5.0, t)
    return w.astype(np.float32)


def _make_in_maps(st, inputs, xT_own, idx_all, pm):
    Wl = [np.asarray(inputs[f'Wl{l}'], np.float32) for l in range(3)]
    Wr = [np.asarray(inputs[f'Wr{l}'], np.float32) for l in range(3)]
    KT = [st['K_A'][b] + st['K_B'][b] for b in range(len(st['K_A']))]
    KTMAX = max(KT) + 1                      # +1 local self slot
    import ml_dtypes
    att_sc = (1.0 + NEG_SLOPE) / 2.0 / 3.0
    attb = []
    wrow = []
    NSH_REAL, NBLK = st['NSH_REAL'], len(st['K_A'])
    PAD_P0 = NSH_REAL - (NBLK - 1) * P
    for l, fe in ((0, 128), (1, 128), (2, 32)):
        a = (np.asarray(inputs[f'att{l}'], np.float32).reshape(-1) * att_sc)
        attb.append(np.ascontiguousarray(
            np.tile(a, (P, KTMAX)).astype(ml_dtypes.bfloat16)))
        w = _poison_row(a, fe)
        wc = np.zeros((P, 128), np.float32)
        wc[PAD_P0:, 0:fe] = w
        wrow.append(wc.astype(ml_dtypes.bfloat16))
    shared = {
        'xT_own': None,
        'W01_0': np.ascontiguousarray(np.concatenate([Wl[0], Wr[0]], 1)).astype(ml_dtypes.bfloat16),
        'W01_1': np.ascontiguousarray(np.concatenate([Wl[1], Wr[1]], 1)).astype(ml_dtypes.bfloat16),
        'W01_2': np.ascontiguousarray(np.concatenate([Wl[2], Wr[2]], 1)).astype(ml_dtypes.bfloat16),
        'attb_0': attb[0], 'attb_1': attb[1], 'attb_2': attb[2],
        'cW1': _blkdiag(np.asarray(inputs['cW1'], np.float32), 3),
        'cW2': _blkdiag(np.asarray(inputs['cW2'], np.float32), 6),
        'ident': np.eye(P, dtype=np.float32).astype(ml_dtypes.bfloat16),
    }
    if not st['trivial']:
        shared.update({
            'bl01_0': _rep(np.concatenate([inputs['bl0'], inputs['br0']])),
            'bl01_1': _rep(np.concatenate([inputs['bl1'], inputs['br1']])),
            'bl01_2': _rep(np.concatenate([inputs['bl2'], inputs['br2']])),
            'g_0': _rep(inputs['g0']), 'g_1': _rep(inputs['g1']),
            'g_2': _rep(inputs['g2']),
            'be_0': _rep(inputs['be0']), 'be_1': _rep(inputs['be1']),
            'be_2': _rep(inputs['be2']),
            'bo_0': _rep(inputs['bo0']), 'bo_1': _rep(inputs['bo1']),
            'bo_2': _rep(inputs['bo2']),
            'cb1': _rep(np.tile(np.asarray(inputs['cb1'], np.float32), 3)),
            'cb2': _rep(inputs['cb2']),
        })
    in_maps = []
    zero_wc = np.zeros((P, 128), ml_dtypes.bfloat16)
    for c in range(NC):
        m = dict(shared)
        m['padmask'] = pm
        m['xT_own'] = xT_own[c].astype(ml_dtypes.bfloat16)
        m['idx_all'] = idx_all[c]
        for l in range(3):
            m[f'wc_{l}'] = wrow[l] if c in (1, 6) else zero_wc
        in_maps.append(m)
    return in_maps


def _check_trivial(inputs):
    zs = ['bl0', 'br0', 'bl1', 'br1', 'bl2', 'br2', 'bo0', 'bo1', 'bo2',
          'be0', 'be1', 'be2', 'cb1', 'cb2']
    on = ['g0', 'g1', 'g2']
    for k in zs:
        if not np.all(np.asarray(inputs[k]) == 0.0):
            return False
    for k in on:
        if not np.all(np.asarray(inputs[k]) == 1.0):
            return False
    return True


_CACHE = {}
_PREP_CACHE = {}


def _run_sim(nc, in_maps):
    from concourse.bass_interp import MultiCoreSim
    sim = MultiCoreSim(nc, num_cores=NC, trace=False,
                       require_finite=False, require_nnan=False)
    cores = list(sim.cores.values())
    for c in range(NC):
        for k, v in in_maps[c].items():
            cores[c].tensor(k)[:] = v
    sim.simulate(check_with_hw=False)
    return [{'out': np.array(cores[c].tensor('out'))} for c in range(NC)]


def kernel(trace=False, backend='hw', **inputs):
    import hashlib
    x = np.asarray(inputs['x'], np.float32)
    ei = np.asarray(inputs['edge_index'])
    pkey = (x.shape, ei.shape,
            hashlib.sha1(np.ascontiguousarray(x)).hexdigest(),
            hashlib.sha1(np.ascontiguousarray(ei)).hexdigest())
    if pkey not in _PREP_CACHE:
        _PREP_CACHE.clear()
        _PREP_CACHE[pkey] = _prep(x, ei)
    st, xT_own, idx_all, row, padmask = _PREP_CACHE[pkey]
    import os as _os
    st = dict(st)
    st['trivial'] = _check_trivial(inputs)
    st['gmax'] = _os.environ.get('GMAX', '7')
    st['scratch'] = _os.environ.get('SCRATCH', '16384')
    st['fuse2'] = _os.environ.get('FUSE2', '1')
    st['nocustom'] = _os.environ.get('NO_CUSTOM', '0')
    st['altw'] = _os.environ.get('ALTW', '1')
    skey = str(sorted((k, str(v)) for k, v in st.items()))
    if skey not in _CACHE:
        _CACHE[skey] = _build(st)
    nc = _CACHE[skey]
    in_maps = _make_in_maps(st, inputs, xT_own, idx_all, padmask)
    if backend == 'sim':
        results = _run_sim(nc, in_maps)
        res = None
    else:
        res = bass_utils.run_bass_kernel_spmd(
            nc, in_maps, core_ids=list(range(NC)), trace=trace)
        results = res.results
    cat = np.concatenate([results[c]['out'] for c in range(NC)])
    out = cat[row]
    if trace:
        kernel.last_results = res
    return out.astype(np.float32)

